# revision 1
# baseline (speedup 1.0000x reference)
"""Trainium2 Bass kernel for the 2-layer heterogeneous GCN encoder.

Strategy (8 NeuronCores, SPMD, dst-sharded):
  - Core k owns user rows [k*12500,(k+1)*12500) and item rows
    [k*6250,(k+1)*6250); edges are routed to their dst owner.
  - Aggregate-then-transform: segment_sum(x[src]*norm, dst) @ W with the
    per-window segment sum done as PE matmuls against an on-chip one-hot
    S[e, col] = (dstcol[e] == col) * norm[e].
  - Tables (x and the layer-1 activations) live in DRAM as fp16; src rows
    are fetched with large batched SWDGE dma_gather calls (int16 indices,
    up to TSEG*128 rows per instruction) instead of per-tile indirect
    DMAs - the per-call SWDGE descriptor-generation overhead dominates
    otherwise.
  - int16 gather indices only span 32768 rows, so each table is swept in
    chunks; per-window partial aggregates accumulate into resident SBUF
    fp16 accumulators across the chunk sweep, then each window is
    transformed (W matmul, bias/relu) and written out. Layer-1 outputs
    AllGather (fp16) across the 8 cores; layer 2 repeats the structure
    reading the gathered tables. Final outputs are written fp32.

Self-contained: hardcodes problem shapes; host does the index-side prep
(degrees/norms, sharding, (chunk,window) sort, int16 packing, fp16 casts).
"""

import sys

sys.path.insert(0, "/opt/trn_rl_repo")

import numpy as np

import concourse.bass as bass
import concourse.bacc as bacc
import concourse.mybir as mybir
import concourse.tile as tile
from concourse.bass_utils import run_bass_kernel_spmd

P = 128
NCORES = 8
F32 = mybir.dt.float32
F16 = mybir.dt.float16
I16 = mybir.dt.int16

CFG = dict(N_U=100000, N_I=50000, E=1600000, D=128)
WIN = 256     # dst rows per aggregation window (psum free size)
CHUNK = 32768  # table rows addressable by one int16 gather index space
TSEG = 8      # tiles (x128 rows) per dma_gather call; num_idxs>1024 hangs
              # the SWDGE ucode on hardware (1024-descriptor ring)

# relation -> (src table, dst type)
RELS = {
    "follows": ("user", "user"),
    "rates": ("user", "item"),
    "rev": ("item", "user"),
}


def _cdiv(a, b):
    return (a + b - 1) // b


def prep_relation(src, dst, n_src, n_dst, ncores=NCORES, win=WIN, chunk=CHUNK,
                  tseg=TSEG):
    """Shard edges by dst owner, sort by (src-chunk, dst-window), pad each
    (chunk,window) run to whole 128-edge tiles harmonized across cores.

    Returns (sched dict, per-core packed arrays [idx16, dstw, norm])."""
    shard = n_dst // ncores
    nwin = _cdiv(shard, win)
    nchunk = _cdiv(n_src, chunk)

    deg_s = np.bincount(src, minlength=n_src).astype(np.float64)
    deg_d = np.bincount(dst, minlength=n_dst).astype(np.float64)
    inv_s = np.where(deg_s > 0, 1.0 / np.sqrt(deg_s), 0.0)
    inv_d = np.where(deg_d > 0, 1.0 / np.sqrt(deg_d), 0.0)
    norm = (inv_s[src] * inv_d[dst]).astype(np.float32)

    owner = dst // shard
    counts = np.zeros((ncores, nchunk, nwin), np.int64)
    per_core = []
    for k in range(ncores):
        sel = owner == k
        s_k = src[sel]
        d_k = dst[sel] - k * shard
        n_k = norm[sel]
        key = (s_k // chunk) * nwin + (d_k // win)
        order = np.argsort(key, kind="stable")
        s_k, d_k, n_k = s_k[order], d_k[order], n_k[order]
        counts[k] = np.bincount(key[order], minlength=nchunk * nwin).reshape(
            nchunk, nwin
        )
        per_core.append((s_k, d_k, n_k))

    Twc = -(-counts.max(axis=0) // P)          # [nchunk, nwin] tiles
    T_c = Twc.sum(axis=1)                       # tiles per chunk
    base_c = np.concatenate([[0], np.cumsum(T_c)[:-1]])
    base_cw = np.zeros((nchunk, nwin), np.int64)
    flat = Twc.reshape(-1)
    base_cw.reshape(-1)[:] = np.concatenate([[0], np.cumsum(flat)[:-1]])
    Ttot = int(T_c.sum())
    Ttot = max(Ttot, 1)

    sched = dict(
        nwin=nwin, nchunk=nchunk, shard=shard,
        Twc=Twc.tolist(), T_c=T_c.tolist(), base_c=base_c.tolist(), Ttot=Ttot,
    )

    packed = []
    for k in range(ncores):
        s_k, d_k, n_k = per_core[k]
        idxw = np.zeros((P, Ttot * 8), np.int16)
        dstw = np.full((P, Ttot), -1.0, np.float32)
        nrm = np.zeros((P, Ttot), np.float32)
        cnt = counts[k]
        starts = np.concatenate([[0], np.cumsum(cnt.ravel())[:-1]])
        tok = np.arange(len(s_k)) - np.repeat(starts, cnt.ravel())
        c_e = s_k // chunk
        w_e = d_k // win
        t_stream = base_cw[c_e, w_e] + tok // P    # global stream tile
        p = tok % P
        dstw[p, t_stream] = (d_k % win).astype(np.float32)
        nrm[p, t_stream] = n_k
        t_loc = t_stream - base_c[c_e]
        j = (t_loc % tseg) * P + p                  # position within segment
        seg = t_loc // tseg
        col = (base_c[c_e] + seg * tseg) * 8 + j // 16
        idxw[j % 16, col] = (s_k - c_e * chunk).astype(np.int16)
        packed.append((idxw, dstw, nrm))
    return sched, packed


def build_program(cfg, scheds, win=WIN, chunk=CHUNK, tseg=TSEG):
    N_U, N_I, D = cfg["N_U"], cfg["N_I"], cfg["D"]
    SU, SI = N_U // NCORES, N_I // NCORES
    NWU, NWI = _cdiv(SU, win), _cdiv(SI, win)

    import os
    ABL_NOS = os.environ.get("ABL_NOS") == "1"        # skip S-builds (DVE)
    ABL_NOMM = os.environ.get("ABL_NOMM") == "1"      # skip tile matmuls (PE)
    ABL_NOGATHER = os.environ.get("ABL_NOGATHER") == "1"  # skip gathers (Pool)

    nc = bacc.Bacc("TRN2", target_bir_lowering=False, num_swdge_queues=4)

    xu16s = nc.dram_tensor("xu16s", [SU, D], F16, kind="ExternalInput")
    xi16s = nc.dram_tensor("xi16s", [SI, D], F16, kind="ExternalInput")
    W16in = {
        n: nc.dram_tensor(f"{n}_h", [D, D], F16, kind="ExternalInput")
        for n in ["W1_follows", "W1_rates", "W1_rev",
                  "W2_follows", "W2_rates", "W2_rev"]
    }
    bs = {
        n: nc.dram_tensor(n, [D], F32, kind="ExternalInput")
        for n in ["b1_follows", "b1_rates", "b1_rev",
                  "b2_follows", "b2_rates", "b2_rev"]
    }
    iota_in = nc.dram_tensor("iota16", [P, win], F16, kind="ExternalInput")
    ident_in = nc.dram_tensor("ident", [P, P], F32, kind="ExternalInput")
    streams = {}
    for r, sc in scheds.items():
        streams[r] = dict(
            idx=nc.dram_tensor(f"idx_{r}", [P, sc["Ttot"] * 8], I16,
                               kind="ExternalInput"),
            dstw=nc.dram_tensor(f"dstw_{r}", [P, sc["Ttot"]], F16,
                                kind="ExternalInput"),
            norm=nc.dram_tensor(f"norm_{r}", [P, sc["Ttot"]], F16,
                                kind="ExternalInput"),
        )
    out_user = nc.dram_tensor("out_user", [SU, D], F16, kind="ExternalOutput")
    out_item = nc.dram_tensor("out_item", [SI, D], F16, kind="ExternalOutput")

    qctr = [0]

    def next_q():
        q = qctr[0] % 4
        qctr[0] += 1
        return q

    with tile.TileContext(nc) as tc:
        with (
            tc.tile_pool(name="const", bufs=1) as cp,
            tc.tile_pool(name="accp", bufs=1) as ap_,
            tc.tile_pool(name="ixp", bufs=3) as ip,
            tc.tile_pool(name="gp", bufs=6) as gp,
            tc.tile_pool(name="Sp", bufs=16) as sp,
            tc.tile_pool(name="hp", bufs=3) as hp,
            tc.tile_pool(name="outp", bufs=4) as outp,
            tc.tile_pool(name="ps", bufs=4, space="PSUM") as pp,
            tc.tile_pool(name="ps2", bufs=2, space="PSUM") as pp2,
            tc.tile_pool(name="pstr", bufs=2, space="PSUM") as ptr,
            tc.tile_pool(name="dram", bufs=1, space="DRAM") as dp,
        ):
            # ---- constants ----
            iota_t = cp.tile([P, win], F16, tag="iota")
            nc.sync.dma_start(iota_t[:], iota_in[:])
            ident_t = cp.tile([P, P], F32, tag="ident")
            nc.sync.dma_start(ident_t[:], ident_in[:])
            W_t = {}
            for n, W in W16in.items():
                W_t[n] = cp.tile([P, P], F16, tag=f"W_{n}", name=f"W_{n}")
                nc.sync.dma_start(W_t[n][:], W[:])
            b_t = {}
            for n, b in bs.items():
                b_t[n] = cp.tile([P, 1], F32, tag=f"b_{n}", name=f"bt_{n}")
                nc.sync.dma_start(b_t[n][:], b[:].unsqueeze(1))
            buv = {}
            for l in (1, 2):
                buv[l] = cp.tile([P, 1], F32, tag=f"b{l}uv", name=f"b{l}uv")
                nc.vector.tensor_tensor(
                    out=buv[l][:], in0=b_t[f"b{l}_follows"][:],
                    in1=b_t[f"b{l}_rev"][:], op=mybir.AluOpType.add,
                )
                nc.vector.tensor_scalar_mul(buv[l][:], buv[l][:], 0.5)

            # ---- dst/norm streams: fp16 input -> f32 resident in SBUF ----
            # (DVE is_equal needs an f32 scalar operand)
            st = {}
            for r, sc in scheds.items():
                st[r] = {}
                for a in ("dstw", "norm"):
                    st16 = cp.tile([P, sc["Ttot"]], F16, tag=f"st16_{r}_{a}",
                                   name=f"st16_{r}_{a}")
                    nc.sync.dma_start(st16[:], streams[r][a][:])
                    st[r][a] = cp.tile([P, sc["Ttot"]], F32, tag=f"{a}_{r}",
                                       name=f"{a}t_{r}")
                    nc.vector.tensor_copy(out=st[r][a][:], in_=st16[:])

            # ---- DRAM tables ----
            # (collectives may not read IO tensors: stage input slices into
            # Internal DRAM first)
            xu_stage = dp.tile([SU, D], F16, tag="xu_stage")
            xi_stage = dp.tile([SI, D], F16, tag="xi_stage")
            nc.sync.dma_start(xu_stage[:], xu16s[:])
            nc.sync.dma_start(xi_stage[:], xi16s[:])
            xu_full = dp.tile([N_U, D], F16, tag="xu_full", addr_space="Shared")
            xi_full = dp.tile([N_I, D], F16, tag="xi_full", addr_space="Shared")
            nc.gpsimd.collective_compute(
                "AllGather", mybir.AluOpType.bypass,
                replica_groups=[list(range(NCORES))],
                ins=[xu_stage[:]], outs=[xu_full[:]],
            )
            nc.gpsimd.collective_compute(
                "AllGather", mybir.AluOpType.bypass,
                replica_groups=[list(range(NCORES))],
                ins=[xi_stage[:]], outs=[xi_full[:]],
            )
            u_slice = dp.tile([SU, D], F16, tag="u_slice")
            it_slice = dp.tile([SI, D], F16, tag="it_slice")
            u_full = dp.tile([N_U, D], F16, tag="u_full", addr_space="Shared")
            it_full = dp.tile([N_I, D], F16, tag="it_full", addr_space="Shared")

            def emit_sweep(rel, table_ap, table_rows, acc, flush_cb=None):
                """Chunk-major edge sweep for one relation; partial window
                aggregates land in `acc` [P, nwin*win] fp16 ([feat, dst])."""
                sc = scheds[rel]
                nchunk, nwin = sc["nchunk"], sc["nwin"]
                Twc, T_c, base_c = sc["Twc"], sc["T_c"], sc["base_c"]
                last_chunk = [
                    max((c for c in range(nchunk) if Twc[c][w] > 0), default=-1)
                    for w in range(nwin)
                ]
                first = [True] * nwin
                Tcmax = max(T_c)
                for c in range(nchunk):
                    if T_c[c] == 0:
                        continue
                    # one idx DMA per chunk; gathers slice it per segment.
                    # idx block is replicated into all 8 16-partition
                    # stripes (one per Q7 subcore).
                    ixb = ip.tile([P, Tcmax * 8], I16, tag="ix", name="ixb")
                    nc.sync.dma_start(
                        ixb[:, : T_c[c] * 8],
                        streams[rel]["idx"][
                            :, base_c[c] * 8 : (base_c[c] + T_c[c]) * 8
                        ],
                    )
                    cur_seg = -1
                    gbuf = None
                    t_cursor = 0
                    for w in range(nwin):
                        nt = Twc[c][w]
                        if nt == 0:
                            continue
                        ps = pp.tile([P, win], F32, tag="runps")
                        for j in range(nt):
                            t_loc = t_cursor + j
                            s = t_loc // tseg
                            if s != cur_seg:
                                cur_seg = s
                                L = min(tseg, T_c[c] - s * tseg)
                                c0 = s * tseg * 8
                                gbuf = gp.tile([P, tseg, P], F16, tag="g")
                                if not ABL_NOGATHER:
                                    nc.gpsimd.dma_gather(
                                        gbuf[:, :L, :],
                                        table_ap[
                                            c * chunk : min((c + 1) * chunk,
                                                            table_rows), :
                                        ],
                                        ixb[:, c0 : c0 + L * 8],
                                        L * P,
                                        L * P,
                                        D,
                                        elem_step=D,
                                        queue_num=next_q(),
                                    )
                            off = t_loc % tseg
                            t_glob = base_c[c] + t_loc
                            if ABL_NOS:
                                S = iota_t
                            else:
                                S = sp.tile([P, win], F16, tag="S")
                                nc.vector.tensor_scalar(
                                    out=S[:],
                                    in0=iota_t[:],
                                    scalar1=st[rel]["dstw"][:, t_glob : t_glob + 1],
                                    scalar2=st[rel]["norm"][:, t_glob : t_glob + 1],
                                    op0=mybir.AluOpType.is_equal,
                                    op1=mybir.AluOpType.mult,
                                )
                            if not ABL_NOMM:
                                nc.tensor.matmul(
                                    out=ps[:],
                                    lhsT=gbuf[:, off, :] if not ABL_NOGATHER
                                    else iota_t[:, :P],
                                    rhs=S[:],
                                    start=(j == 0),
                                    stop=(j == nt - 1),
                                )
                        wsl = acc[:, w * win : (w + 1) * win]
                        if first[w]:
                            nc.scalar.activation(
                                out=wsl, in_=ps[:],
                                func=mybir.ActivationFunctionType.Copy,
                            )
                            first[w] = False
                        else:
                            nc.vector.tensor_tensor(
                                out=wsl, in0=ps[:], in1=wsl,
                                op=mybir.AluOpType.add,
                            )
                        if flush_cb is not None and c == last_chunk[w]:
                            flush_cb(w)
                        t_cursor += nt
                for w in range(nwin):
                    if last_chunk[w] < 0:
                        nc.vector.memset(acc[:, w * win : (w + 1) * win], 0.0)
                        if flush_cb is not None:
                            flush_cb(w)

            def write_block(h, dst_ap, w, nrows, dt):
                """h [P(feat), win] f32 -> transpose -> dst rows (dtype dt)."""
                for blk in range(_cdiv(nrows, P)):
                    r0, r1 = blk * P, min((blk + 1) * P, nrows)
                    pt = ptr.tile([P, P], F32, tag="ptr")
                    nc.tensor.transpose(
                        out=pt[: r1 - r0, :], in_=h[:, r0:r1],
                        identity=ident_t[:],
                    )
                    ob = outp.tile([P, P], dt, tag="ob")
                    nc.scalar.activation(
                        out=ob[: r1 - r0, :], in_=pt[: r1 - r0, :],
                        func=mybir.ActivationFunctionType.Copy,
                    )
                    nc.sync.dma_start(
                        dst_ap[w * win + r0 : w * win + r1, :], ob[: r1 - r0, :]
                    )

            def make_user_flush(l, accF, accV, dst_ap, dt):
                Wf, Wv = W_t[f"W{l}_follows"], W_t[f"W{l}_rev"]
                bias = buv[l]

                def flush(w):
                    nrows = min(win, SU - w * win)
                    wsl = slice(w * win, (w + 1) * win)
                    ph = pp2.tile([P, win], F32, tag="phps")
                    nc.tensor.matmul(out=ph[:], lhsT=Wf[:], rhs=accF[:, wsl],
                                     start=True, stop=False)
                    nc.tensor.matmul(out=ph[:], lhsT=Wv[:], rhs=accV[:, wsl],
                                     start=False, stop=True)
                    h = hp.tile([P, win], F32, tag="h")
                    if l == 1:
                        nc.scalar.activation(
                            out=h[:], in_=ph[:],
                            func=mybir.ActivationFunctionType.Relu,
                            bias=bias[:], scale=0.5,
                        )
                    else:
                        nc.vector.tensor_scalar(
                            out=h[:], in0=ph[:], scalar1=0.5, scalar2=bias[:],
                            op0=mybir.AluOpType.mult, op1=mybir.AluOpType.add,
                        )
                    write_block(h, dst_ap, w, nrows, dt)

                return flush

            def make_item_flush(l, accR, dst_ap, dt):
                Wr = W_t[f"W{l}_rates"]
                bias = b_t[f"b{l}_rates"]

                def flush(w):
                    nrows = min(win, SI - w * win)
                    wsl = slice(w * win, (w + 1) * win)
                    ph = pp2.tile([P, win], F32, tag="phps")
                    nc.tensor.matmul(out=ph[:], lhsT=Wr[:], rhs=accR[:, wsl],
                                     start=True, stop=True)
                    h = hp.tile([P, win], F32, tag="h")
                    if l == 1:
                        nc.scalar.activation(
                            out=h[:], in_=ph[:],
                            func=mybir.ActivationFunctionType.Relu,
                            bias=bias[:], scale=1.0,
                        )
                    else:
                        nc.vector.tensor_scalar(
                            out=h[:], in0=ph[:], scalar1=1.0, scalar2=bias[:],
                            op0=mybir.AluOpType.mult, op1=mybir.AluOpType.add,
                        )
                    write_block(h, dst_ap, w, nrows, dt)

                return flush

            def user_pass(l, tabU, rowsU, tabI, rowsI, dst_ap, dt):
                accF = ap_.tile([P, NWU * win], F16, tag="accF")
                accV = ap_.tile([P, NWU * win], F16, tag="accV")
                emit_sweep("follows", tabU, rowsU, accF)
                emit_sweep("rev", tabI, rowsI, accV,
                           flush_cb=make_user_flush(l, accF, accV, dst_ap, dt))

            def item_pass(l, tabU, rowsU, dst_ap, dt):
                accR = ap_.tile([P, NWI * win], F16, tag="accR")
                emit_sweep("rates", tabU, rowsU, accR,
                           flush_cb=make_item_flush(l, accR, dst_ap, dt))

            # ---- layer 1 ----
            user_pass(1, xu_full, N_U, xi_full, N_I, u_slice, F16)
            nc.gpsimd.collective_compute(
                "AllGather", mybir.AluOpType.bypass,
                replica_groups=[list(range(NCORES))],
                ins=[u_slice[:]], outs=[u_full[:]],
            )
            item_pass(1, xu_full, N_U, it_slice, F16)
            nc.gpsimd.collective_compute(
                "AllGather", mybir.AluOpType.bypass,
                replica_groups=[list(range(NCORES))],
                ins=[it_slice[:]], outs=[it_full[:]],
            )
            # ---- layer 2 (rates first: only needs u_full) ----
            item_pass(2, u_full, N_U, out_item.ap(), F16)
            user_pass(2, u_full, N_U, it_full, N_I, out_user.ap(), F16)

    nc.compile()
    return nc


def prepare(inputs, cfg=None, win=WIN, chunk=CHUNK, tseg=TSEG):
    """Host-side prep + program build. Returns (nc, in_maps)."""
    if cfg is None:
        cfg = dict(CFG)
    N_U = inputs["x_user"].shape[0]
    N_I = inputs["x_item"].shape[0]
    cfg.update(N_U=N_U, N_I=N_I, E=len(inputs["follows_src"]))

    rel_edges = {
        "follows": (inputs["follows_src"], inputs["follows_dst"], N_U, N_U),
        "rates": (inputs["rates_src"], inputs["rates_dst"], N_U, N_I),
        "rev": (inputs["rev_src"], inputs["rev_dst"], N_I, N_U),
    }
    scheds, packs = {}, {}
    for r, (s, d, ns, nd) in rel_edges.items():
        sched, packed = prep_relation(
            np.asarray(s), np.asarray(d), ns, nd,
            win=win, chunk=chunk, tseg=tseg,
        )
        scheds[r] = sched
        packs[r] = packed

    nc = build_program(cfg, scheds, win=win, chunk=chunk, tseg=tseg)

    iota16 = np.broadcast_to(
        np.arange(win, dtype=np.float16), (P, win)
    ).copy()
    ident = np.eye(P, dtype=np.float32)
    xu16 = np.asarray(inputs["x_user"]).astype(np.float16)
    xi16 = np.asarray(inputs["x_item"]).astype(np.float16)
    SU, SI = cfg["N_U"] // NCORES, cfg["N_I"] // NCORES
    common = {
        "iota16": iota16,
        "ident": ident,
    }
    for n in ["W1_follows", "W1_rates", "W1_rev",
              "W2_follows", "W2_rates", "W2_rev"]:
        common[f"{n}_h"] = np.asarray(inputs[n]).astype(np.float16)
    for n in ["b1_follows", "b1_rates", "b1_rev",
              "b2_follows", "b2_rates", "b2_rev"]:
        common[n] = np.asarray(inputs[n])
    in_maps = []
    for k in range(NCORES):
        m = dict(common)
        m["xu16s"] = xu16[k * SU : (k + 1) * SU]
        m["xi16s"] = xi16[k * SI : (k + 1) * SI]
        for r in rel_edges:
            idxw, dstw, nrm = packs[r][k]
            m[f"idx_{r}"] = np.tile(idxw[:16], (8, 1))
            m[f"dstw_{r}"] = dstw.astype(np.float16)
            m[f"norm_{r}"] = nrm.astype(np.float16)
        in_maps.append(m)
    return nc, in_maps


def assemble(results):
    u2 = np.concatenate([results[k]["out_user"] for k in range(NCORES)], axis=0)
    i2 = np.concatenate([results[k]["out_item"] for k in range(NCORES)], axis=0)
    return np.concatenate([u2, i2], axis=0).astype(np.float32)


def kernel(**inputs):
    nc, in_maps = prepare(inputs)
    res = run_bass_kernel_spmd(nc, in_maps, list(range(NCORES)))
    return assemble(res.results)


if __name__ == "__main__":
    pass



# revision 5
# speedup vs baseline: 1.1803x; 1.1803x over previous
"""Trainium2 Bass kernel for the 2-layer heterogeneous GCN encoder.

v3 strategy (8 NeuronCores, SPMD, dst-sharded):
  - Core k owns user rows [k*12500,(k+1)*12500) and item rows
    [k*6250,(k+1)*6250); edges are routed to their dst owner.
  - inv_s (src-degree norm) is folded into the gathered tables: layer-1
    tables are host-prescaled replicated fp16 inputs (one per relation);
    layer-1 flushes write pre-scaled fp16 tables for layer 2, which are
    AllGathered across cores.
  - inv_d (dst-degree norm), the 0.5 relation-mean and the bias are folded
    into the flush via scalar_tensor_tensor on [dst,feat] blocks, computed
    without PE transposes by swapping matmul operands
    (out_blk = agg_blk^T @ W with agg as lhsT).
  - Aggregation: segment_sum(x~[src], dst) per window accumulated in PSUM
    per (chunk, window), added into a resident fp16 SBUF acc; the one-hot
    S[e, col] = (dstcol[e] == col) is built 4 tiles at a time with a
    single stride-0-broadcast DVE is_equal (no norm multiply).
  - src rows fetched with 1024-row SWDGE dma_gather segments (int16
    indices; >1024 idx/call wedges the SWDGE ucode).

Self-contained: hardcodes shapes; host does index prep (degrees, sharding,
(chunk,window) sort, int16 packing, fp16 casts, table prescaling).
"""

import os
import sys

sys.path.insert(0, "/opt/trn_rl_repo")

import numpy as np

import concourse.bass as bass
import concourse.bacc as bacc
import concourse.mybir as mybir
import concourse.tile as tile
from concourse.bass_utils import run_bass_kernel_spmd

P = 128
NCORES = 8
F32 = mybir.dt.float32
F16 = mybir.dt.float16
I16 = mybir.dt.int16

CFG = dict(N_U=100000, N_I=50000, E=1600000, D=128)
WIN = 256      # dst rows per aggregation window (<=512 f32 psum bank)
CHUNK = 32768  # table rows addressable by one int16 gather index space
TSEG = 8       # tiles (x128 rows) per dma_gather call (1024-idx ucode cap)
SGRP = 4       # tiles per S-build group
NQ = 4         # SWDGE queues

# relation -> (src type, dst type)
RELS = {
    "follows": ("user", "user"),
    "rates": ("user", "item"),
    "rev": ("item", "user"),
}


def _cdiv(a, b):
    return (a + b - 1) // b


def prep_relation(src, dst, n_src, n_dst, ncores=NCORES, win=WIN, chunk=CHUNK,
                  tseg=TSEG):
    """Shard edges by dst owner, sort by (src-chunk, dst-window), pad each
    (chunk,window) run to whole 128-edge tiles harmonized across cores.

    Returns (sched dict, per-core [idx16, dstw16], inv_s, inv_d)."""
    shard = n_dst // ncores
    nwin = _cdiv(shard, win)
    nchunk = _cdiv(n_src, chunk)

    deg_s = np.bincount(src, minlength=n_src).astype(np.float64)
    deg_d = np.bincount(dst, minlength=n_dst).astype(np.float64)
    inv_s = np.where(deg_s > 0, 1.0 / np.sqrt(deg_s), 0.0).astype(np.float32)
    inv_d = np.where(deg_d > 0, 1.0 / np.sqrt(deg_d), 0.0).astype(np.float32)

    owner = dst // shard
    counts = np.zeros((ncores, nchunk, nwin), np.int64)
    per_core = []
    for k in range(ncores):
        sel = owner == k
        s_k = src[sel]
        d_k = dst[sel] - k * shard
        key = (s_k // chunk) * nwin + (d_k // win)
        order = np.argsort(key, kind="stable")
        s_k, d_k = s_k[order], d_k[order]
        counts[k] = np.bincount(key[order], minlength=nchunk * nwin).reshape(
            nchunk, nwin
        )
        per_core.append((s_k, d_k))

    Twc = -(-counts.max(axis=0) // P)          # [nchunk, nwin] tiles
    T_c = Twc.sum(axis=1)                       # tiles per chunk
    base_c = np.concatenate([[0], np.cumsum(T_c)[:-1]])
    base_cw = np.zeros((nchunk, nwin), np.int64)
    flat = Twc.reshape(-1)
    base_cw.reshape(-1)[:] = np.concatenate([[0], np.cumsum(flat)[:-1]])
    Ttot = max(int(T_c.sum()), 1)

    sched = dict(
        nwin=nwin, nchunk=nchunk, shard=shard,
        Twc=Twc.tolist(), T_c=T_c.tolist(), base_c=base_c.tolist(), Ttot=Ttot,
    )

    packed = []
    for k in range(ncores):
        s_k, d_k = per_core[k]
        idxw = np.zeros((16, Ttot * 8), np.int16)
        dstw = np.full((P, Ttot), -1.0, np.float32)
        cnt = counts[k]
        starts = np.concatenate([[0], np.cumsum(cnt.ravel())[:-1]])
        tok = np.arange(len(s_k)) - np.repeat(starts, cnt.ravel())
        c_e = s_k // chunk
        w_e = d_k // win
        t_stream = base_cw[c_e, w_e] + tok // P    # global stream tile
        p = tok % P
        dstw[p, t_stream] = (d_k % win).astype(np.float32)
        t_loc = t_stream - base_c[c_e]
        j = (t_loc % tseg) * P + p                  # position within segment
        seg = t_loc // tseg
        col = (base_c[c_e] + seg * tseg) * 8 + j // 16
        idxw[j % 16, col] = (s_k - c_e * chunk).astype(np.int16)
        packed.append((np.tile(idxw, (8, 1)), dstw))
    return sched, packed, inv_s, inv_d


def _pack_pcol(v, nb):
    """[rows] -> [P, nb] with col b, part p = v[b*128+p] (zero-padded)."""
    out = np.zeros(nb * P, np.float32)
    out[: len(v)] = v
    return out.reshape(nb, P).T.copy()


def build_program(cfg, scheds, win=WIN, chunk=CHUNK, tseg=TSEG, sgrp=SGRP):
    N_U, N_I, D = cfg["N_U"], cfg["N_I"], cfg["D"]
    SU, SI = N_U // NCORES, N_I // NCORES
    NWU, NWI = _cdiv(SU, win), _cdiv(SI, win)
    NBU, NBI = _cdiv(SU, P), _cdiv(SI, P)
    BLKW = win // P  # 128-blocks per window (win % P == 0)
    assert win % P == 0

    ABL_NOS = os.environ.get("ABL_NOS") == "1"
    ABL_NOGATHER = os.environ.get("ABL_NOGATHER") == "1"

    nc = bacc.Bacc("TRN2", target_bir_lowering=False, num_swdge_queues=NQ)

    # ---- replicated full fp16 tables (host-prescaled by inv_s) ----
    xuf_in = nc.dram_tensor("xuf", [N_U, D], F16, kind="ExternalInput")
    xur_in = nc.dram_tensor("xur", [N_U, D], F16, kind="ExternalInput")
    xiv_in = nc.dram_tensor("xiv", [N_I, D], F16, kind="ExternalInput")
    W16in = {
        n: nc.dram_tensor(f"{n}_h", [D, D], F16, kind="ExternalInput")
        for n in ["W1_follows", "W1_rates", "W1_rev",
                  "W2_follows", "W2_rates", "W2_rev"]
    }
    brep_in = {
        n: nc.dram_tensor(n, [P, D], F32, kind="ExternalInput")
        for n in ["b1uv", "b1r", "b2uv", "b2r"]
    }
    # per-core packed [P, nb] consts
    pc_in = {
        "invdFh": nc.dram_tensor("invdFh", [P, NBU], F32, kind="ExternalInput"),
        "invdVh": nc.dram_tensor("invdVh", [P, NBU], F32, kind="ExternalInput"),
        "invdR": nc.dram_tensor("invdR", [P, NBI], F32, kind="ExternalInput"),
        "invsF": nc.dram_tensor("invsF", [P, NBU], F32, kind="ExternalInput"),
        "invsR": nc.dram_tensor("invsR", [P, NBU], F32, kind="ExternalInput"),
        "invsV": nc.dram_tensor("invsV", [P, NBI], F32, kind="ExternalInput"),
    }
    iota_in = nc.dram_tensor("iota16", [P, win], F16, kind="ExternalInput")
    streams = {}
    for r, sc in scheds.items():
        streams[r] = dict(
            idx=nc.dram_tensor(f"idx_{r}", [P, sc["Ttot"] * 8], I16,
                               kind="ExternalInput"),
            dstw=nc.dram_tensor(f"dstw_{r}", [P, sc["Ttot"]], F32,
                                kind="ExternalInput"),
        )
    out_user = nc.dram_tensor("out_user", [SU, D], F16, kind="ExternalOutput")
    out_item = nc.dram_tensor("out_item", [SI, D], F16, kind="ExternalOutput")

    qctr = [0]

    def next_q():
        q = qctr[0] % NQ
        qctr[0] += 1
        return q

    with tile.TileContext(nc) as tc:
        with (
            tc.tile_pool(name="const", bufs=1) as cp,
            tc.tile_pool(name="accp", bufs=1) as ap_,
            tc.tile_pool(name="ixp", bufs=2) as ip,
            tc.tile_pool(name="gp", bufs=8) as gp,
            tc.tile_pool(name="Sp", bufs=16) as sp,
            tc.tile_pool(name="hp", bufs=6) as hp,
            tc.tile_pool(name="outp", bufs=8) as outp,
            tc.tile_pool(name="ps", bufs=4, space="PSUM") as pp,
            tc.tile_pool(name="ps2", bufs=4, space="PSUM") as pp2,
            tc.tile_pool(name="dram", bufs=1, space="DRAM") as dp,
        ):
            # ---- constants ----
            iota_t = cp.tile([P, win], F16, tag="iota")
            nc.sync.dma_start(iota_t[:], iota_in[:])
            W_t = {}
            for n, W in W16in.items():
                W_t[n] = cp.tile([P, P], F16, tag=f"W_{n}", name=f"W_{n}")
                nc.sync.dma_start(W_t[n][:], W[:])
            brep_t = {}
            for n, b in brep_in.items():
                brep_t[n] = cp.tile([P, D], F32, tag=f"br_{n}", name=f"br_{n}")
                nc.sync.dma_start(brep_t[n][:], b[:])
            pc_t = {}
            for n, t in pc_in.items():
                nb = NBI if n.endswith(("R", "V")) and n != "invsR" else NBU
                nb = t.shape[1]
                pc_t[n] = cp.tile([P, nb], F32, tag=f"pc_{n}", name=f"pc_{n}")
                nc.sync.dma_start(pc_t[n][:], t[:])
            st = {}
            for r, sc in scheds.items():
                st[r] = cp.tile([P, sc["Ttot"]], F32, tag=f"dstw_{r}",
                                name=f"dstw_t_{r}")
                nc.sync.dma_start(st[r][:], streams[r]["dstw"][:])
            zc = cp.tile([P, win], F16, tag="zc")
            nc.vector.memset(zc[:], 0.0)

            # ---- DRAM layer-2 tables ----
            uf_slice = dp.tile([SU, D], F16, tag="uf_slice")
            ur_slice = dp.tile([SU, D], F16, tag="ur_slice")
            itv_slice = dp.tile([SI, D], F16, tag="itv_slice")
            uf_full = dp.tile([N_U, D], F16, tag="uf_full", addr_space="Shared")
            ur_full = dp.tile([N_U, D], F16, tag="ur_full", addr_space="Shared")
            itv_full = dp.tile([N_I, D], F16, tag="itv_full",
                               addr_space="Shared")

            def emit_sweep(rel, table_ap, table_rows, acc, flush_cb=None):
                """(c,w)-major edge sweep for one relation; window partial
                aggregates accumulate into `acc` [P, nwin*win] fp16."""
                sc = scheds[rel]
                nchunk, nwin = sc["nchunk"], sc["nwin"]
                Twc, T_c, base_c = sc["Twc"], sc["T_c"], sc["base_c"]
                last_chunk = [
                    max((c for c in range(nchunk) if Twc[c][w] > 0), default=-1)
                    for w in range(nwin)
                ]
                first = [True] * nwin
                Tcmax = max(T_c)
                for c in range(nchunk):
                    if T_c[c] == 0:
                        continue
                    ixb = ip.tile([P, Tcmax * 8], I16, tag="ix", name="ixb")
                    nc.sync.dma_start(
                        ixb[:, : T_c[c] * 8],
                        streams[rel]["idx"][
                            :, base_c[c] * 8 : (base_c[c] + T_c[c]) * 8
                        ],
                    )
                    cur_seg = -1
                    gbuf = None
                    t_cursor = 0
                    for w in range(nwin):
                        nt = Twc[c][w]
                        if nt == 0:
                            continue
                        ps = pp.tile([P, win], F32, tag="runps")
                        for j in range(nt):
                            t_loc = t_cursor + j
                            s = t_loc // tseg
                            if s != cur_seg:
                                cur_seg = s
                                L = min(tseg, T_c[c] - s * tseg)
                                c0 = s * tseg * 8
                                gbuf = gp.tile([P, tseg, P], F16, tag="g")
                                if not ABL_NOGATHER:
                                    nc.gpsimd.dma_gather(
                                        gbuf[:, :L, :],
                                        table_ap[
                                            c * chunk : min((c + 1) * chunk,
                                                            table_rows), :
                                        ],
                                        ixb[:, c0 : c0 + L * 8],
                                        L * P,
                                        L * P,
                                        D,
                                        elem_step=D,
                                        queue_num=next_q(),
                                    )
                            t_glob = base_c[c] + t_loc
                            if ABL_NOS:
                                Sg = iota_t
                            else:
                                Sg = sp.tile([P, win], F16, tag="S")
                                nc.vector.tensor_scalar(
                                    out=Sg[:],
                                    in0=iota_t[:],
                                    scalar1=st[rel][:, t_glob : t_glob + 1],
                                    scalar2=None,
                                    op0=mybir.AluOpType.is_equal,
                                )
                            nc.tensor.matmul(
                                out=ps[:],
                                lhsT=gbuf[:, t_loc % tseg, :]
                                if not ABL_NOGATHER else iota_t[:, :P],
                                rhs=Sg[:],
                                start=(j == 0),
                                stop=(j == nt - 1),
                            )
                        wsl = acc[:, w * win : (w + 1) * win]
                        if first[w]:
                            nc.scalar.activation(
                                out=wsl, in_=ps[:],
                                func=mybir.ActivationFunctionType.Copy,
                            )
                            first[w] = False
                        else:
                            nc.vector.tensor_tensor(
                                out=wsl, in0=ps[:], in1=wsl,
                                op=mybir.AluOpType.add,
                            )
                        if flush_cb is not None and c == last_chunk[w]:
                            flush_cb(w)
                        t_cursor += nt
                for w in range(nwin):
                    if last_chunk[w] < 0:
                        nc.vector.memset(acc[:, w * win : (w + 1) * win], 0.0)
                        if flush_cb is not None:
                            flush_cb(w)

            def make_user_flush(l, accF, accV, shard_rows):
                """User flush: out[d,:] = invd_f/2*(aggF@Wf) + invd_v/2*(aggV@Wv)
                + 0.5(bf+bv); layer1 -> relu -> 2 prescaled fp16 tables."""
                Wf, Wv = W_t[f"W{l}_follows"], W_t[f"W{l}_rev"]
                brep = brep_t["b1uv" if l == 1 else "b2uv"]

                def flush(w):
                    nrows = min(win, shard_rows - w * win)
                    for b in range(_cdiv(nrows, P)):
                        nr = min(P, nrows - b * P)
                        col = w * BLKW + b
                        cs = slice(b * P, b * P + nr)
                        phF = pp2.tile([P, P], F32, tag="ph")
                        nc.tensor.matmul(
                            out=phF[:nr, :],
                            lhsT=accF[:, w * win : (w + 1) * win][:, cs],
                            rhs=Wf[:], start=True, stop=True,
                        )
                        phV = pp2.tile([P, P], F32, tag="ph")
                        nc.tensor.matmul(
                            out=phV[:nr, :],
                            lhsT=accV[:, w * win : (w + 1) * win][:, cs],
                            rhs=Wv[:], start=True, stop=True,
                        )
                        t1 = hp.tile([P, D], F32, tag="t1")
                        nc.vector.scalar_tensor_tensor(
                            out=t1[:nr, :], in0=phF[:nr, :],
                            scalar=pc_t["invdFh"][:nr, col : col + 1],
                            in1=brep[:nr, :],
                            op0=mybir.AluOpType.mult,
                            op1=mybir.AluOpType.add,
                        )
                        r0 = w * win + b * P
                        if l == 1:
                            t2 = hp.tile([P, D], F32, tag="t2")
                            nc.vector.scalar_tensor_tensor(
                                out=t2[:nr, :], in0=phV[:nr, :],
                                scalar=pc_t["invdVh"][:nr, col : col + 1],
                                in1=t1[:nr, :],
                                op0=mybir.AluOpType.mult,
                                op1=mybir.AluOpType.add,
                            )
                            u16 = outp.tile([P, D], F16, tag="u16")
                            nc.scalar.activation(
                                out=u16[:nr, :], in_=t2[:nr, :],
                                func=mybir.ActivationFunctionType.Relu,
                            )
                            ufo = outp.tile([P, D], F16, tag="ufo")
                            nc.vector.tensor_scalar(
                                out=ufo[:nr, :], in0=u16[:nr, :],
                                scalar1=pc_t["invsF"][:nr, col : col + 1],
                                scalar2=None, op0=mybir.AluOpType.mult,
                            )
                            nc.sync.dma_start(
                                uf_slice[r0 : r0 + nr, :], ufo[:nr, :]
                            )
                            uro = outp.tile([P, D], F16, tag="uro")
                            nc.vector.tensor_scalar(
                                out=uro[:nr, :], in0=u16[:nr, :],
                                scalar1=pc_t["invsR"][:nr, col : col + 1],
                                scalar2=None, op0=mybir.AluOpType.mult,
                            )
                            nc.sync.dma_start(
                                ur_slice[r0 : r0 + nr, :], uro[:nr, :]
                            )
                        else:
                            o16 = outp.tile([P, D], F16, tag="o16")
                            nc.vector.scalar_tensor_tensor(
                                out=o16[:nr, :], in0=phV[:nr, :],
                                scalar=pc_t["invdVh"][:nr, col : col + 1],
                                in1=t1[:nr, :],
                                op0=mybir.AluOpType.mult,
                                op1=mybir.AluOpType.add,
                            )
                            nc.sync.dma_start(
                                out_user.ap()[r0 : r0 + nr, :], o16[:nr, :]
                            )

                return flush

            def make_item_flush(l, accR, shard_rows):
                Wr = W_t[f"W{l}_rates"]
                brep = brep_t["b1r" if l == 1 else "b2r"]

                def flush(w):
                    nrows = min(win, shard_rows - w * win)
                    for b in range(_cdiv(nrows, P)):
                        nr = min(P, nrows - b * P)
                        col = w * BLKW + b
                        cs = slice(b * P, b * P + nr)
                        phR = pp2.tile([P, P], F32, tag="ph")
                        nc.tensor.matmul(
                            out=phR[:nr, :],
                            lhsT=accR[:, w * win : (w + 1) * win][:, cs],
                            rhs=Wr[:], start=True, stop=True,
                        )
                        r0 = w * win + b * P
                        if l == 1:
                            t1 = hp.tile([P, D], F32, tag="t1")
                            nc.vector.scalar_tensor_tensor(
                                out=t1[:nr, :], in0=phR[:nr, :],
                                scalar=pc_t["invdR"][:nr, col : col + 1],
                                in1=brep[:nr, :],
                                op0=mybir.AluOpType.mult,
                                op1=mybir.AluOpType.add,
                            )
                            i16 = outp.tile([P, D], F16, tag="i16")
                            nc.scalar.activation(
                                out=i16[:nr, :], in_=t1[:nr, :],
                                func=mybir.ActivationFunctionType.Relu,
                            )
                            ivo = outp.tile([P, D], F16, tag="ivo")
                            nc.vector.tensor_scalar(
                                out=ivo[:nr, :], in0=i16[:nr, :],
                                scalar1=pc_t["invsV"][:nr, col : col + 1],
                                scalar2=None, op0=mybir.AluOpType.mult,
                            )
                            nc.sync.dma_start(
                                itv_slice[r0 : r0 + nr, :], ivo[:nr, :]
                            )
                        else:
                            o16 = outp.tile([P, D], F16, tag="o16")
                            nc.vector.scalar_tensor_tensor(
                                out=o16[:nr, :], in0=phR[:nr, :],
                                scalar=pc_t["invdR"][:nr, col : col + 1],
                                in1=brep[:nr, :],
                                op0=mybir.AluOpType.mult,
                                op1=mybir.AluOpType.add,
                            )
                            nc.sync.dma_start(
                                out_item.ap()[r0 : r0 + nr, :], o16[:nr, :]
                            )

                return flush

            def user_pass(l, tabU, rowsU, tabI, rowsI):
                accF = ap_.tile([P, NWU * win], F16, tag="accF")
                accV = ap_.tile([P, NWU * win], F16, tag="accV")
                emit_sweep("follows", tabU, rowsU, accF)
                emit_sweep("rev", tabI, rowsI, accV,
                           flush_cb=make_user_flush(l, accF, accV, SU))

            def item_pass(l, tabU, rowsU):
                accR = ap_.tile([P, NWI * win], F16, tag="accR")
                emit_sweep("rates", tabU, rowsU, accR,
                           flush_cb=make_item_flush(l, accR, SI))

            # ---- layer 1 ----
            user_pass(1, xuf_in.ap(), N_U, xiv_in.ap(), N_I)
            nc.gpsimd.collective_compute(
                "AllGather", mybir.AluOpType.bypass,
                replica_groups=[list(range(NCORES))],
                ins=[ur_slice[:]], outs=[ur_full[:]],
            )
            nc.gpsimd.collective_compute(
                "AllGather", mybir.AluOpType.bypass,
                replica_groups=[list(range(NCORES))],
                ins=[uf_slice[:]], outs=[uf_full[:]],
            )
            item_pass(1, xur_in.ap(), N_U)
            nc.gpsimd.collective_compute(
                "AllGather", mybir.AluOpType.bypass,
                replica_groups=[list(range(NCORES))],
                ins=[itv_slice[:]], outs=[itv_full[:]],
            )
            # ---- layer 2 (rates first: only needs ur_full) ----
            item_pass(2, ur_full, N_U)
            user_pass(2, uf_full, N_U, itv_full, N_I)

    nc.compile()
    return nc


def prepare(inputs, cfg=None, win=WIN, chunk=CHUNK, tseg=TSEG):
    """Host-side prep + program build. Returns (nc, in_maps)."""
    if cfg is None:
        cfg = dict(CFG)
    N_U = inputs["x_user"].shape[0]
    N_I = inputs["x_item"].shape[0]
    cfg.update(N_U=N_U, N_I=N_I, E=len(inputs["follows_src"]))
    SU, SI = N_U // NCORES, N_I // NCORES
    NBU, NBI = _cdiv(SU, P), _cdiv(SI, P)

    rel_edges = {
        "follows": (inputs["follows_src"], inputs["follows_dst"], N_U, N_U),
        "rates": (inputs["rates_src"], inputs["rates_dst"], N_U, N_I),
        "rev": (inputs["rev_src"], inputs["rev_dst"], N_I, N_U),
    }
    scheds, packs, invs, invd = {}, {}, {}, {}
    for r, (s, d, ns, nd) in rel_edges.items():
        sched, packed, inv_s, inv_d = prep_relation(
            np.asarray(s), np.asarray(d), ns, nd, win=win, chunk=chunk,
            tseg=tseg,
        )
        scheds[r] = sched
        packs[r] = packed
        invs[r] = inv_s
        invd[r] = inv_d

    nc = build_program(cfg, scheds, win=win, chunk=chunk, tseg=tseg)

    xu = np.asarray(inputs["x_user"]).astype(np.float32)
    xi = np.asarray(inputs["x_item"]).astype(np.float32)
    common = {
        "xuf": (xu * invs["follows"][:, None]).astype(np.float16),
        "xur": (xu * invs["rates"][:, None]).astype(np.float16),
        "xiv": (xi * invs["rev"][:, None]).astype(np.float16),
        "iota16": np.broadcast_to(
            np.arange(win, dtype=np.float16), (P, win)
        ).copy(),
    }
    for n in ["W1_follows", "W1_rates", "W1_rev",
              "W2_follows", "W2_rates", "W2_rev"]:
        common[f"{n}_h"] = np.asarray(inputs[n]).astype(np.float16)
    b = {n: np.asarray(inputs[n]).astype(np.float32)
         for n in ["b1_follows", "b1_rates", "b1_rev",
                   "b2_follows", "b2_rates", "b2_rev"]}
    common["b1uv"] = np.tile(
        (0.5 * (b["b1_follows"] + b["b1_rev"]))[None, :], (P, 1))
    common["b2uv"] = np.tile(
        (0.5 * (b["b2_follows"] + b["b2_rev"]))[None, :], (P, 1))
    common["b1r"] = np.tile(b["b1_rates"][None, :], (P, 1))
    common["b2r"] = np.tile(b["b2_rates"][None, :], (P, 1))

    in_maps = []
    for k in range(NCORES):
        m = dict(common)
        us = slice(k * SU, (k + 1) * SU)
        its = slice(k * SI, (k + 1) * SI)
        m["invdFh"] = _pack_pcol(0.5 * invd["follows"][us], NBU)
        m["invdVh"] = _pack_pcol(0.5 * invd["rev"][us], NBU)
        m["invdR"] = _pack_pcol(invd["rates"][its], NBI)
        m["invsF"] = _pack_pcol(invs["follows"][us], NBU)
        m["invsR"] = _pack_pcol(invs["rates"][us], NBU)
        m["invsV"] = _pack_pcol(invs["rev"][its], NBI)
        for r in rel_edges:
            idxw, dstw = packs[r][k]
            m[f"idx_{r}"] = idxw
            m[f"dstw_{r}"] = dstw
        in_maps.append(m)
    return nc, in_maps


def assemble(results):
    u2 = np.concatenate([results[k]["out_user"] for k in range(NCORES)], axis=0)
    i2 = np.concatenate([results[k]["out_item"] for k in range(NCORES)], axis=0)
    return np.concatenate([u2, i2], axis=0).astype(np.float32)


def kernel(**inputs):
    nc, in_maps = prepare(inputs)
    res = run_bass_kernel_spmd(nc, in_maps, list(range(NCORES)))
    return assemble(res.results)


if __name__ == "__main__":
    pass


# revision 6
# speedup vs baseline: 1.2284x; 1.0408x over previous
"""Trainium2 Bass kernel for the 2-layer heterogeneous GCN encoder.

v3 strategy (8 NeuronCores, SPMD, dst-sharded):
  - Core k owns user rows [k*12500,(k+1)*12500) and item rows
    [k*6250,(k+1)*6250); edges are routed to their dst owner.
  - inv_s (src-degree norm) is folded into the gathered tables: layer-1
    tables are host-prescaled replicated fp16 inputs (one per relation);
    layer-1 flushes write pre-scaled fp16 tables for layer 2, which are
    AllGathered across cores.
  - inv_d (dst-degree norm), the 0.5 relation-mean and the bias are folded
    into the flush via scalar_tensor_tensor on [dst,feat] blocks, computed
    without PE transposes by swapping matmul operands
    (out_blk = agg_blk^T @ W with agg as lhsT).
  - Aggregation: segment_sum(x~[src], dst) per window accumulated in PSUM
    per (chunk, window), added into a resident fp16 SBUF acc; the one-hot
    S[e, col] = (dstcol[e] == col) is built 4 tiles at a time with a
    single stride-0-broadcast DVE is_equal (no norm multiply).
  - src rows fetched with 1024-row SWDGE dma_gather segments (int16
    indices; >1024 idx/call wedges the SWDGE ucode).

Self-contained: hardcodes shapes; host does index prep (degrees, sharding,
(chunk,window) sort, int16 packing, fp16 casts, table prescaling).
"""

import os
import sys

sys.path.insert(0, "/opt/trn_rl_repo")

import numpy as np

import concourse.bass as bass
import concourse.bacc as bacc
import concourse.mybir as mybir
import concourse.tile as tile
from concourse.bass_utils import run_bass_kernel_spmd

P = 128
NCORES = 8
F32 = mybir.dt.float32
F16 = mybir.dt.float16
I16 = mybir.dt.int16

CFG = dict(N_U=100000, N_I=50000, E=1600000, D=128)
WIN = 256      # dst rows per aggregation window (<=512 f32 psum bank)
CHUNK = 32768  # table rows addressable by one int16 gather index space
TSEG = 8       # tiles (x128 rows) per dma_gather call (1024-idx ucode cap)
SGRP = 4       # tiles per S-build group
NQ = 4         # SWDGE queues

# relation -> (src type, dst type)
RELS = {
    "follows": ("user", "user"),
    "rates": ("user", "item"),
    "rev": ("item", "user"),
}


def _cdiv(a, b):
    return (a + b - 1) // b


def prep_relation(src, dst, n_src, n_dst, ncores=NCORES, win=WIN, chunk=CHUNK,
                  tseg=TSEG):
    """Shard edges by dst owner, sort by (src-chunk, dst-window), pad each
    (chunk,window) run to whole 128-edge tiles harmonized across cores.

    Returns (sched dict, per-core [idx16, dstw16], inv_s, inv_d)."""
    shard = n_dst // ncores
    nwin = _cdiv(shard, win)
    nchunk = _cdiv(n_src, chunk)

    deg_s = np.bincount(src, minlength=n_src).astype(np.float64)
    deg_d = np.bincount(dst, minlength=n_dst).astype(np.float64)
    inv_s = np.where(deg_s > 0, 1.0 / np.sqrt(deg_s), 0.0).astype(np.float32)
    inv_d = np.where(deg_d > 0, 1.0 / np.sqrt(deg_d), 0.0).astype(np.float32)

    owner = dst // shard
    counts = np.zeros((ncores, nchunk, nwin), np.int64)
    per_core = []
    for k in range(ncores):
        sel = owner == k
        s_k = src[sel]
        d_k = dst[sel] - k * shard
        key = (s_k // chunk) * nwin + (d_k // win)
        order = np.argsort(key, kind="stable")
        s_k, d_k = s_k[order], d_k[order]
        counts[k] = np.bincount(key[order], minlength=nchunk * nwin).reshape(
            nchunk, nwin
        )
        per_core.append((s_k, d_k))

    Twc = -(-counts.max(axis=0) // P)          # [nchunk, nwin] tiles
    T_c = Twc.sum(axis=1)                       # tiles per chunk
    base_c = np.concatenate([[0], np.cumsum(T_c)[:-1]])
    base_cw = np.zeros((nchunk, nwin), np.int64)
    flat = Twc.reshape(-1)
    base_cw.reshape(-1)[:] = np.concatenate([[0], np.cumsum(flat)[:-1]])
    Ttot = max(int(T_c.sum()), 1)

    sched = dict(
        nwin=nwin, nchunk=nchunk, shard=shard,
        Twc=Twc.tolist(), T_c=T_c.tolist(), base_c=base_c.tolist(), Ttot=Ttot,
    )

    packed = []
    for k in range(ncores):
        s_k, d_k = per_core[k]
        idxw = np.zeros((16, Ttot * 8), np.int16)
        dstw = np.full((P, Ttot), -1.0, np.float32)
        cnt = counts[k]
        starts = np.concatenate([[0], np.cumsum(cnt.ravel())[:-1]])
        tok = np.arange(len(s_k)) - np.repeat(starts, cnt.ravel())
        c_e = s_k // chunk
        w_e = d_k // win
        t_stream = base_cw[c_e, w_e] + tok // P    # global stream tile
        p = tok % P
        dstw[p, t_stream] = (d_k % win).astype(np.float32)
        t_loc = t_stream - base_c[c_e]
        j = (t_loc % tseg) * P + p                  # position within segment
        seg = t_loc // tseg
        col = (base_c[c_e] + seg * tseg) * 8 + j // 16
        idxw[j % 16, col] = (s_k - c_e * chunk).astype(np.int16)
        packed.append((np.tile(idxw, (8, 1)), dstw))
    return sched, packed, inv_s, inv_d


def _pack_pcol(v, nb):
    """[rows] -> [P, nb] with col b, part p = v[b*128+p] (zero-padded)."""
    out = np.zeros(nb * P, np.float32)
    out[: len(v)] = v
    return out.reshape(nb, P).T.copy()


def build_program(cfg, scheds, win=WIN, chunk=CHUNK, tseg=TSEG, sgrp=SGRP):
    N_U, N_I, D = cfg["N_U"], cfg["N_I"], cfg["D"]
    SU, SI = N_U // NCORES, N_I // NCORES
    NWU, NWI = _cdiv(SU, win), _cdiv(SI, win)
    NBU, NBI = _cdiv(SU, P), _cdiv(SI, P)
    BLKW = win // P  # 128-blocks per window (win % P == 0)
    assert win % P == 0

    ABL_NOS = os.environ.get("ABL_NOS") == "1"
    ABL_NOGATHER = os.environ.get("ABL_NOGATHER") == "1"

    nc = bacc.Bacc("TRN2", target_bir_lowering=False, num_swdge_queues=NQ)

    # ---- replicated full fp16 tables (host-prescaled by inv_s) ----
    xuf_in = nc.dram_tensor("xuf", [N_U, D], F16, kind="ExternalInput")
    xur_in = nc.dram_tensor("xur", [N_U, D], F16, kind="ExternalInput")
    xiv_in = nc.dram_tensor("xiv", [N_I, D], F16, kind="ExternalInput")
    W16in = {
        n: nc.dram_tensor(f"{n}_h", [D, D], F16, kind="ExternalInput")
        for n in ["W1_follows", "W1_rates", "W1_rev",
                  "W2_follows", "W2_rates", "W2_rev"]
    }
    brep_in = {
        n: nc.dram_tensor(n, [P, D], F32, kind="ExternalInput")
        for n in ["b1uv", "b1r", "b2uv", "b2r"]
    }
    # per-core packed [P, nb] consts
    pc_in = {
        "invdFh": nc.dram_tensor("invdFh", [P, NBU], F32, kind="ExternalInput"),
        "invdVh": nc.dram_tensor("invdVh", [P, NBU], F32, kind="ExternalInput"),
        "invdR": nc.dram_tensor("invdR", [P, NBI], F32, kind="ExternalInput"),
        "invsF": nc.dram_tensor("invsF", [P, NBU], F32, kind="ExternalInput"),
        "invsR": nc.dram_tensor("invsR", [P, NBU], F32, kind="ExternalInput"),
        "invsV": nc.dram_tensor("invsV", [P, NBI], F32, kind="ExternalInput"),
    }
    iota_in = nc.dram_tensor("iota16", [P, win], F16, kind="ExternalInput")
    ident_in = nc.dram_tensor("identF", [P, P], F16, kind="ExternalInput")
    streams = {}
    for r, sc in scheds.items():
        streams[r] = dict(
            idx=nc.dram_tensor(f"idx_{r}", [P, sc["Ttot"] * 8], I16,
                               kind="ExternalInput"),
            dstw=nc.dram_tensor(f"dstw_{r}", [P, sc["Ttot"]], F32,
                                kind="ExternalInput"),
        )
    out_user = nc.dram_tensor("out_user", [SU, D], F16, kind="ExternalOutput")
    out_item = nc.dram_tensor("out_item", [SI, D], F16, kind="ExternalOutput")

    qctr = [0]

    def next_q():
        q = qctr[0] % NQ
        qctr[0] += 1
        return q

    with tile.TileContext(nc) as tc:
        with (
            tc.tile_pool(name="const", bufs=1) as cp,
            tc.tile_pool(name="accp", bufs=1) as ap_,
            tc.tile_pool(name="ixp", bufs=2) as ip,
            tc.tile_pool(name="gp", bufs=10) as gp,
            tc.tile_pool(name="Sp", bufs=24) as sp,
            tc.tile_pool(name="hp", bufs=6) as hp,
            tc.tile_pool(name="outp", bufs=8) as outp,
            tc.tile_pool(name="ps", bufs=4, space="PSUM") as pp,
            tc.tile_pool(name="ps2", bufs=4, space="PSUM") as pp2,
            tc.tile_pool(name="dram", bufs=1, space="DRAM") as dp,
        ):
            # ---- constants ----
            iota_t = cp.tile([P, win], F16, tag="iota")
            nc.sync.dma_start(iota_t[:], iota_in[:])
            ident_t = cp.tile([P, P], F16, tag="identF")
            nc.sync.dma_start(ident_t[:], ident_in[:])
            W_t = {}
            for n, W in W16in.items():
                W_t[n] = cp.tile([P, P], F16, tag=f"W_{n}", name=f"W_{n}")
                nc.sync.dma_start(W_t[n][:], W[:])
            brep_t = {}
            for n, b in brep_in.items():
                brep_t[n] = cp.tile([P, D], F32, tag=f"br_{n}", name=f"br_{n}")
                nc.sync.dma_start(brep_t[n][:], b[:])
            pc_t = {}
            for n, t in pc_in.items():
                nb = NBI if n.endswith(("R", "V")) and n != "invsR" else NBU
                nb = t.shape[1]
                pc_t[n] = cp.tile([P, nb], F32, tag=f"pc_{n}", name=f"pc_{n}")
                nc.sync.dma_start(pc_t[n][:], t[:])
            st = {}
            for r, sc in scheds.items():
                st[r] = cp.tile([P, sc["Ttot"]], F32, tag=f"dstw_{r}",
                                name=f"dstw_t_{r}")
                nc.sync.dma_start(st[r][:], streams[r]["dstw"][:])
            zc = cp.tile([P, win], F16, tag="zc")
            nc.vector.memset(zc[:], 0.0)

            # ---- DRAM layer-2 tables ----
            uf_slice = dp.tile([SU, D], F16, tag="uf_slice")
            ur_slice = dp.tile([SU, D], F16, tag="ur_slice")
            itv_slice = dp.tile([SI, D], F16, tag="itv_slice")
            uf_full = dp.tile([N_U, D], F16, tag="uf_full", addr_space="Shared")
            ur_full = dp.tile([N_U, D], F16, tag="ur_full", addr_space="Shared")
            itv_full = dp.tile([N_I, D], F16, tag="itv_full",
                               addr_space="Shared")

            def emit_sweep(rel, table_ap, table_rows, acc, flush_cb=None):
                """(c,w)-major edge sweep for one relation; window partial
                aggregates accumulate into `acc` [P, nwin*win] fp16."""
                sc = scheds[rel]
                nchunk, nwin = sc["nchunk"], sc["nwin"]
                Twc, T_c, base_c = sc["Twc"], sc["T_c"], sc["base_c"]
                last_chunk = [
                    max((c for c in range(nchunk) if Twc[c][w] > 0), default=-1)
                    for w in range(nwin)
                ]
                first = [True] * nwin
                Tcmax = max(T_c)
                for c in range(nchunk):
                    if T_c[c] == 0:
                        continue
                    ixb = ip.tile([P, Tcmax * 8], I16, tag="ix", name="ixb")
                    nc.gpsimd.dma_start(
                        ixb[:, : T_c[c] * 8],
                        streams[rel]["idx"][
                            :, base_c[c] * 8 : (base_c[c] + T_c[c]) * 8
                        ],
                    )
                    cur_seg = -1
                    gbuf = None
                    t_cursor = 0
                    for w in range(nwin):
                        nt = Twc[c][w]
                        if nt == 0:
                            continue
                        ps = pp.tile([P, win], F32, tag="runps")
                        wsl = acc[:, w * win : (w + 1) * win]
                        if not first[w]:
                            # re-inject prior partial into the fresh psum
                            # chain on PE (keeps acc updates off the DVE)
                            nc.tensor.matmul(
                                out=ps[:], lhsT=ident_t[:], rhs=wsl,
                                start=True, stop=False,
                            )
                        for j in range(nt):
                            t_loc = t_cursor + j
                            s = t_loc // tseg
                            if s != cur_seg:
                                cur_seg = s
                                L = min(tseg, T_c[c] - s * tseg)
                                c0 = s * tseg * 8
                                gbuf = gp.tile([P, tseg, P], F16, tag="g")
                                if not ABL_NOGATHER:
                                    nc.gpsimd.dma_gather(
                                        gbuf[:, :L, :],
                                        table_ap[
                                            c * chunk : min((c + 1) * chunk,
                                                            table_rows), :
                                        ],
                                        ixb[:, c0 : c0 + L * 8],
                                        L * P,
                                        L * P,
                                        D,
                                        elem_step=D,
                                        queue_num=next_q(),
                                    )
                            t_glob = base_c[c] + t_loc
                            if ABL_NOS:
                                Sg = iota_t
                            else:
                                Sg = sp.tile([P, win], F16, tag="S")
                                nc.vector.tensor_scalar(
                                    out=Sg[:],
                                    in0=iota_t[:],
                                    scalar1=st[rel][:, t_glob : t_glob + 1],
                                    scalar2=None,
                                    op0=mybir.AluOpType.is_equal,
                                )
                            nc.tensor.matmul(
                                out=ps[:],
                                lhsT=gbuf[:, t_loc % tseg, :]
                                if not ABL_NOGATHER else iota_t[:, :P],
                                rhs=Sg[:],
                                start=(j == 0 and first[w]),
                                stop=(j == nt - 1),
                            )
                        first[w] = False
                        nc.scalar.activation(
                            out=wsl, in_=ps[:],
                            func=mybir.ActivationFunctionType.Copy,
                        )
                        if flush_cb is not None and c == last_chunk[w]:
                            flush_cb(w)
                        t_cursor += nt
                for w in range(nwin):
                    if last_chunk[w] < 0:
                        nc.vector.memset(acc[:, w * win : (w + 1) * win], 0.0)
                        if flush_cb is not None:
                            flush_cb(w)

            def make_user_flush(l, accF, accV, shard_rows):
                """User flush: out[d,:] = invd_f/2*(aggF@Wf) + invd_v/2*(aggV@Wv)
                + 0.5(bf+bv); layer1 -> relu -> 2 prescaled fp16 tables."""
                Wf, Wv = W_t[f"W{l}_follows"], W_t[f"W{l}_rev"]
                brep = brep_t["b1uv" if l == 1 else "b2uv"]

                def flush(w):
                    nrows = min(win, shard_rows - w * win)
                    for b in range(_cdiv(nrows, P)):
                        nr = min(P, nrows - b * P)
                        col = w * BLKW + b
                        cs = slice(b * P, b * P + nr)
                        phF = pp2.tile([P, P], F32, tag="ph")
                        nc.tensor.matmul(
                            out=phF[:nr, :],
                            lhsT=accF[:, w * win : (w + 1) * win][:, cs],
                            rhs=Wf[:], start=True, stop=True,
                        )
                        phV = pp2.tile([P, P], F32, tag="ph")
                        nc.tensor.matmul(
                            out=phV[:nr, :],
                            lhsT=accV[:, w * win : (w + 1) * win][:, cs],
                            rhs=Wv[:], start=True, stop=True,
                        )
                        t1 = hp.tile([P, D], F32, tag="t1")
                        nc.vector.scalar_tensor_tensor(
                            out=t1[:nr, :], in0=phF[:nr, :],
                            scalar=pc_t["invdFh"][:nr, col : col + 1],
                            in1=brep[:nr, :],
                            op0=mybir.AluOpType.mult,
                            op1=mybir.AluOpType.add,
                        )
                        r0 = w * win + b * P
                        if l == 1:
                            t2 = hp.tile([P, D], F32, tag="t2")
                            nc.vector.scalar_tensor_tensor(
                                out=t2[:nr, :], in0=phV[:nr, :],
                                scalar=pc_t["invdVh"][:nr, col : col + 1],
                                in1=t1[:nr, :],
                                op0=mybir.AluOpType.mult,
                                op1=mybir.AluOpType.add,
                            )
                            u16 = outp.tile([P, D], F16, tag="u16")
                            nc.scalar.activation(
                                out=u16[:nr, :], in_=t2[:nr, :],
                                func=mybir.ActivationFunctionType.Relu,
                            )
                            ufo = outp.tile([P, D], F16, tag="ufo")
                            nc.vector.tensor_scalar(
                                out=ufo[:nr, :], in0=u16[:nr, :],
                                scalar1=pc_t["invsF"][:nr, col : col + 1],
                                scalar2=None, op0=mybir.AluOpType.mult,
                            )
                            nc.sync.dma_start(
                                uf_slice[r0 : r0 + nr, :], ufo[:nr, :]
                            )
                            uro = outp.tile([P, D], F16, tag="uro")
                            nc.vector.tensor_scalar(
                                out=uro[:nr, :], in0=u16[:nr, :],
                                scalar1=pc_t["invsR"][:nr, col : col + 1],
                                scalar2=None, op0=mybir.AluOpType.mult,
                            )
                            nc.sync.dma_start(
                                ur_slice[r0 : r0 + nr, :], uro[:nr, :]
                            )
                        else:
                            o16 = outp.tile([P, D], F16, tag="o16")
                            nc.vector.scalar_tensor_tensor(
                                out=o16[:nr, :], in0=phV[:nr, :],
                                scalar=pc_t["invdVh"][:nr, col : col + 1],
                                in1=t1[:nr, :],
                                op0=mybir.AluOpType.mult,
                                op1=mybir.AluOpType.add,
                            )
                            nc.sync.dma_start(
                                out_user.ap()[r0 : r0 + nr, :], o16[:nr, :]
                            )

                return flush

            def make_item_flush(l, accR, shard_rows):
                Wr = W_t[f"W{l}_rates"]
                brep = brep_t["b1r" if l == 1 else "b2r"]

                def flush(w):
                    nrows = min(win, shard_rows - w * win)
                    for b in range(_cdiv(nrows, P)):
                        nr = min(P, nrows - b * P)
                        col = w * BLKW + b
                        cs = slice(b * P, b * P + nr)
                        phR = pp2.tile([P, P], F32, tag="ph")
                        nc.tensor.matmul(
                            out=phR[:nr, :],
                            lhsT=accR[:, w * win : (w + 1) * win][:, cs],
                            rhs=Wr[:], start=True, stop=True,
                        )
                        r0 = w * win + b * P
                        if l == 1:
                            t1 = hp.tile([P, D], F32, tag="t1")
                            nc.vector.scalar_tensor_tensor(
                                out=t1[:nr, :], in0=phR[:nr, :],
                                scalar=pc_t["invdR"][:nr, col : col + 1],
                                in1=brep[:nr, :],
                                op0=mybir.AluOpType.mult,
                                op1=mybir.AluOpType.add,
                            )
                            i16 = outp.tile([P, D], F16, tag="i16")
                            nc.scalar.activation(
                                out=i16[:nr, :], in_=t1[:nr, :],
                                func=mybir.ActivationFunctionType.Relu,
                            )
                            ivo = outp.tile([P, D], F16, tag="ivo")
                            nc.vector.tensor_scalar(
                                out=ivo[:nr, :], in0=i16[:nr, :],
                                scalar1=pc_t["invsV"][:nr, col : col + 1],
                                scalar2=None, op0=mybir.AluOpType.mult,
                            )
                            nc.sync.dma_start(
                                itv_slice[r0 : r0 + nr, :], ivo[:nr, :]
                            )
                        else:
                            o16 = outp.tile([P, D], F16, tag="o16")
                            nc.vector.scalar_tensor_tensor(
                                out=o16[:nr, :], in0=phR[:nr, :],
                                scalar=pc_t["invdR"][:nr, col : col + 1],
                                in1=brep[:nr, :],
                                op0=mybir.AluOpType.mult,
                                op1=mybir.AluOpType.add,
                            )
                            nc.sync.dma_start(
                                out_item.ap()[r0 : r0 + nr, :], o16[:nr, :]
                            )

                return flush

            def user_pass(l, tabU, rowsU, tabI, rowsI):
                accF = ap_.tile([P, NWU * win], F16, tag="accF")
                accV = ap_.tile([P, NWU * win], F16, tag="accV")
                emit_sweep("follows", tabU, rowsU, accF)
                emit_sweep("rev", tabI, rowsI, accV,
                           flush_cb=make_user_flush(l, accF, accV, SU))

            def item_pass(l, tabU, rowsU):
                accR = ap_.tile([P, NWI * win], F16, tag="accR")
                emit_sweep("rates", tabU, rowsU, accR,
                           flush_cb=make_item_flush(l, accR, SI))

            # ---- layer 1 ----
            user_pass(1, xuf_in.ap(), N_U, xiv_in.ap(), N_I)
            nc.gpsimd.collective_compute(
                "AllGather", mybir.AluOpType.bypass,
                replica_groups=[list(range(NCORES))],
                ins=[ur_slice[:]], outs=[ur_full[:]],
            )
            nc.gpsimd.collective_compute(
                "AllGather", mybir.AluOpType.bypass,
                replica_groups=[list(range(NCORES))],
                ins=[uf_slice[:]], outs=[uf_full[:]],
            )
            item_pass(1, xur_in.ap(), N_U)
            nc.gpsimd.collective_compute(
                "AllGather", mybir.AluOpType.bypass,
                replica_groups=[list(range(NCORES))],
                ins=[itv_slice[:]], outs=[itv_full[:]],
            )
            # ---- layer 2 (rates first: only needs ur_full) ----
            item_pass(2, ur_full, N_U)
            user_pass(2, uf_full, N_U, itv_full, N_I)

    nc.compile()
    return nc


def prepare(inputs, cfg=None, win=WIN, chunk=CHUNK, tseg=TSEG):
    """Host-side prep + program build. Returns (nc, in_maps)."""
    if cfg is None:
        cfg = dict(CFG)
    N_U = inputs["x_user"].shape[0]
    N_I = inputs["x_item"].shape[0]
    cfg.update(N_U=N_U, N_I=N_I, E=len(inputs["follows_src"]))
    SU, SI = N_U // NCORES, N_I // NCORES
    NBU, NBI = _cdiv(SU, P), _cdiv(SI, P)

    rel_edges = {
        "follows": (inputs["follows_src"], inputs["follows_dst"], N_U, N_U),
        "rates": (inputs["rates_src"], inputs["rates_dst"], N_U, N_I),
        "rev": (inputs["rev_src"], inputs["rev_dst"], N_I, N_U),
    }
    scheds, packs, invs, invd = {}, {}, {}, {}
    for r, (s, d, ns, nd) in rel_edges.items():
        sched, packed, inv_s, inv_d = prep_relation(
            np.asarray(s), np.asarray(d), ns, nd, win=win, chunk=chunk,
            tseg=tseg,
        )
        scheds[r] = sched
        packs[r] = packed
        invs[r] = inv_s
        invd[r] = inv_d

    nc = build_program(cfg, scheds, win=win, chunk=chunk, tseg=tseg)

    xu = np.asarray(inputs["x_user"]).astype(np.float32)
    xi = np.asarray(inputs["x_item"]).astype(np.float32)
    common = {
        "xuf": (xu * invs["follows"][:, None]).astype(np.float16),
        "xur": (xu * invs["rates"][:, None]).astype(np.float16),
        "xiv": (xi * invs["rev"][:, None]).astype(np.float16),
        "iota16": np.broadcast_to(
            np.arange(win, dtype=np.float16), (P, win)
        ).copy(),
        "identF": np.eye(P, dtype=np.float16),
    }
    for n in ["W1_follows", "W1_rates", "W1_rev",
              "W2_follows", "W2_rates", "W2_rev"]:
        common[f"{n}_h"] = np.asarray(inputs[n]).astype(np.float16)
    b = {n: np.asarray(inputs[n]).astype(np.float32)
         for n in ["b1_follows", "b1_rates", "b1_rev",
                   "b2_follows", "b2_rates", "b2_rev"]}
    common["b1uv"] = np.tile(
        (0.5 * (b["b1_follows"] + b["b1_rev"]))[None, :], (P, 1))
    common["b2uv"] = np.tile(
        (0.5 * (b["b2_follows"] + b["b2_rev"]))[None, :], (P, 1))
    common["b1r"] = np.tile(b["b1_rates"][None, :], (P, 1))
    common["b2r"] = np.tile(b["b2_rates"][None, :], (P, 1))

    in_maps = []
    for k in range(NCORES):
        m = dict(common)
        us = slice(k * SU, (k + 1) * SU)
        its = slice(k * SI, (k + 1) * SI)
        m["invdFh"] = _pack_pcol(0.5 * invd["follows"][us], NBU)
        m["invdVh"] = _pack_pcol(0.5 * invd["rev"][us], NBU)
        m["invdR"] = _pack_pcol(invd["rates"][its], NBI)
        m["invsF"] = _pack_pcol(invs["follows"][us], NBU)
        m["invsR"] = _pack_pcol(invs["rates"][us], NBU)
        m["invsV"] = _pack_pcol(invs["rev"][its], NBI)
        for r in rel_edges:
            idxw, dstw = packs[r][k]
            m[f"idx_{r}"] = idxw
            m[f"dstw_{r}"] = dstw
        in_maps.append(m)
    return nc, in_maps


def assemble(results):
    u2 = np.concatenate([results[k]["out_user"] for k in range(NCORES)], axis=0)
    i2 = np.concatenate([results[k]["out_item"] for k in range(NCORES)], axis=0)
    return np.concatenate([u2, i2], axis=0).astype(np.float32)


def kernel(**inputs):
    nc, in_maps = prepare(inputs)
    res = run_bass_kernel_spmd(nc, in_maps, list(range(NCORES)))
    return assemble(res.results)


if __name__ == "__main__":
    pass
